# revision 28
# baseline (speedup 1.0000x reference)
"""Trainium2 Bass kernel for windowed (inverted-window) attention.

Problem: B=2, T=2048, C=2048, H=16 heads, D=128, WINDOW=512.
  q,k,v = x@Wq, x@Wk, x@Wv  (per-head reshape), RoPE on q,k,
  scores masked so positions INSIDE the causal window are masked out
  (attend only to j>i or j<i-511), softmax, o@Wo.

Sharding: 8 cores = 2 (batch) x 4 (head groups of 4 heads).
Each core computes its batch's 4 heads end-to-end plus a partial
output projection (row-chunk of Wo); host sums the 4 partials per batch.

Matmul operands are bf16 (fp32 PSUM accumulation); everything else fp32.
"""

import sys
import numpy as np

for _p in ("/opt/trn_rl_repo",):
    if _p not in sys.path:
        sys.path.insert(0, _p)

import ml_dtypes  # noqa: E402

# If BASS_TRACE is set in the environment, run_bass_kernel_spmd imports
# antenv.axon_hooks, which this container does not ship. Register a stub
# so tracing degrades gracefully instead of crashing.
try:
    import antenv.axon_hooks  # noqa: F401
except ImportError:
    import types as _types

    _hooks = _types.ModuleType("antenv.axon_hooks")
    _hooks._hook = None
    _hooks.set_axon_ntff_profile_hook = lambda h: setattr(_hooks, "_hook", h)
    _hooks.get_axon_ntff_profile_hook = lambda: _hooks._hook
    sys.modules["antenv.axon_hooks"] = _hooks
    import antenv as _antenv

    _antenv.axon_hooks = _hooks
import concourse.bass as bass  # noqa: E402
import concourse.mybir as mybir  # noqa: E402
from concourse.bacc import Bacc  # noqa: E402
from concourse.tile import TileContext  # noqa: E402
from concourse.bass import ts, ds  # noqa: E402
from concourse.bass_utils import run_bass_kernel_spmd  # noqa: E402

B, T, C, H, D = 2, 2048, 2048, 16, 128
HL = 4                # heads per core
NCORES = 8
WINDOW = 512
ROPE_BASE = 10000.0
TB = 512              # i/t block size (matmul free dim)
NTB = T // TB         # 4
CK = C // 128         # 16 contraction chunks for projections
NTC = T // 128        # 16 j-chunks / t-chunks
MASK_OFF = 1920       # master strip offset: off = i0 - j0 + MASK_OFF
MASK_W = 4352
F32 = mybir.dt.float32
BF16 = mybir.dt.bfloat16
AF = mybir.ActivationFunctionType

MM_DT = BF16          # dtype of every matmul operand tensor
NP_MM = ml_dtypes.bfloat16

_NC = None
TRACE = False
LAST_RESULT = None    # BassKernelResults of the most recent run (for test.py)


def build_nc():
    nc = Bacc()
    xT = nc.declare_dram_parameter("xT", [C, T], MM_DT, isOutput=False)
    wq = nc.declare_dram_parameter("wq", [C, HL * D], MM_DT, isOutput=False)
    wk = nc.declare_dram_parameter("wk", [C, HL * D], MM_DT, isOutput=False)
    wv = nc.declare_dram_parameter("wv", [C, HL * D], MM_DT, isOutput=False)
    wo = nc.declare_dram_parameter("wo", [HL * D, C], MM_DT, isOutput=False)
    cosx = nc.declare_dram_parameter("cosx", [128, T], F32, isOutput=False)
    sinx = nc.declare_dram_parameter("sinx", [128, T], F32, isOutput=False)
    maskm = nc.declare_dram_parameter("maskm", [128, MASK_W], MM_DT, isOutput=False)
    out = nc.declare_dram_parameter("out", [T, C], F32, isOutput=True)

    xT_v = xT[:].rearrange("(co p) t -> p co t", p=128)   # [128, 16, T]
    wq_v = wq[:].rearrange("(co p) d -> p co d", p=128)   # [128, 16, 512]
    wk_v = wk[:].rearrange("(co p) d -> p co d", p=128)
    wv_v = wv[:].rearrange("(co p) d -> p co d", p=128)
    wo_v = wo[:].rearrange("(h p) c -> p h c", p=128)     # [128, 4, C]

    scale = float(1.0 / np.sqrt(D))

    with TileContext(nc) as tc:
        with (
            tc.tile_pool(name="res", bufs=1) as res,      # long-lived residents
            tc.tile_pool(name="xbp", bufs=20) as xbp,     # streamed x chunks
            tc.tile_pool(name="ropet", bufs=2) as ropet,
            tc.tile_pool(name="ropes", bufs=2) as ropes,
            tc.tile_pool(name="etp", bufs=17) as etp,
            tc.tile_pool(name="smp", bufs=2) as smp,
            tc.tile_pool(name="zp", bufs=9) as zp,
            tc.tile_pool(name="wop", bufs=2) as wop,
            tc.tile_pool(name="ocb", bufs=2) as ocb,
            tc.tile_pool(name="psum", bufs=1, space="PSUM") as psum,
        ):
            # ---- long-lived tensors; all big loads issued up-front ----
            wqs, wks = [], []
            for ck in range(CK):
                wqc = res.tile([128, HL * D], MM_DT, tag=f"wq{ck}", name=f"wq{ck}")
                nc.sync.dma_start(wqc[:], wq_v[:, ck, :])
                wkc = res.tile([128, HL * D], MM_DT, tag=f"wk{ck}", name=f"wk{ck}")
                nc.sync.dma_start(wkc[:], wk_v[:, ck, :])
                wqs.append(wqc)
                wks.append(wkc)
            wvt = res.tile([128, CK, HL * D], MM_DT)
            nc.sync.dma_start(wvt[:], wv_v[:])
            cosb = res.tile([128, T], F32)
            nc.sync.dma_start(cosb[:], cosx[:])
            sinb = res.tile([128, T], F32)
            nc.sync.dma_start(sinb[:], sinx[:])

            QT = res.tile([128, HL, T], MM_DT)    # q transposed [d, t]
            KT = res.tile([128, HL, T], MM_DT)
            V = res.tile([128, NTC, HL * D], MM_DT)   # v natural [t, hd]
            oT = res.tile([128, HL, T], MM_DT)    # per-head o transposed [d, t]

            # ---- Phase A: projections (QK transposed + RoPE, V natural) ----
            for tb in range(NTB):
                xbs = []
                for ck in range(CK):
                    xb = xbp.tile([128, TB], MM_DT, tag="xtb", name=f"xb{tb}_{ck}")
                    nc.gpsimd.dma_start(xb[:], xT_v[:, ck, ts(tb, TB)])
                    xbs.append(xb)
                pss = []
                for i in range(2 * HL):
                    pst = psum.tile([128, TB], F32, tag=f"pq{i}", name=f"pst{i}")
                    pss.append(pst)
                for ck in range(CK):
                    i = 0
                    for h in range(HL):
                        for wt in (wqs[ck], wks[ck]):
                            nc.tensor.matmul(
                                pss[i][:], wt[:, ts(h, D)], xbs[ck][:],
                                start=(ck == 0), stop=(ck == CK - 1),
                            )
                            i += 1
                i = 0
                for h in range(HL):
                    for OUTT in (QT, KT):
                        ps = pss[i]
                        i += 1
                        # RoPE: out = raw*cos + swap(raw)*sin_signed
                        raw = ropet.tile([128, TB], F32, tag="raw")
                        nc.scalar.copy(raw[:], ps[:])
                        sw = ropes.tile([128, TB], F32, tag="sw")
                        nc.sync.dma_start(sw[0:64, :], raw[64:128, :])
                        nc.sync.dma_start(sw[64:128, :], raw[0:64, :])
                        nc.vector.tensor_mul(sw[:], sw[:], sinb[:, ts(tb, TB)])
                        nc.vector.tensor_mul(raw[:], raw[:], cosb[:, ts(tb, TB)])
                        nc.vector.tensor_add(OUTT[:, h, ts(tb, TB)], sw[:], raw[:])
                # V for the 4 t-chunks of this t-block (reuses the x chunks)
                for tco in range(NTB):
                    tch = tb * NTB + tco
                    psv = psum.tile(
                        [128, HL * D], F32, tag=f"pq{tco}", name=f"psv{tch}"
                    )
                    for ck in range(CK):
                        nc.tensor.matmul(
                            psv[:], xbs[ck][:, ts(tco, 128)], wvt[:, ck, :],
                            start=(ck == 0), stop=(ck == CK - 1),
                        )
                    nc.scalar.copy(V[:, tch, :], psv[:])

            # deferred loads (first needed by attention)
            maskb = res.tile([128, MASK_W], MM_DT)
            nc.sync.dma_start(maskb[:], maskm[:])
            ones = res.tile([128, 128], MM_DT)
            nc.vector.memset(ones[:], 1.0)

            # ---- Phase B: attention + interleaved output projection ----
            for ib in range(NTB):
                for h in range(HL):
                    ets = []
                    for c in range(NTC):
                        ps = psum.tile(
                            [128, TB], F32, tag=f"pq{c % 4}", name=f"pss{h}_{ib}_{c}"
                        )
                        nc.tensor.matmul(
                            ps[:], KT[:, h, ts(c, 128)], QT[:, h, ts(ib, TB)],
                            start=True, stop=True,
                        )
                        et = etp.tile([128, TB], MM_DT, tag="et")
                        nc.scalar.activation(et[:], ps[:], AF.Exp, scale=scale)
                        dd = ib * TB - c * 128
                        if -(WINDOW - 1) <= dd <= (WINDOW - 1) + 127:
                            off = dd + MASK_OFF
                            nc.vector.tensor_mul(et[:], et[:], maskb[:, ds(off, TB)])
                        ets.append(et)
                    pso = psum.tile([128, TB], F32, tag="pq4", name=f"pso{h}_{ib}")
                    psz = psum.tile([128, TB], F32, tag="pq5", name=f"psz{h}_{ib}")
                    us = []
                    for k in range(NTC // 2):
                        u = zp.tile([128, TB], MM_DT, tag=f"u{k}", name=f"u{h}_{ib}_{k}")
                        nc.vector.tensor_add(u[:], ets[2 * k][:], ets[2 * k + 1][:])
                        us.append(u)
                    for c in range(NTC):
                        nc.tensor.matmul(
                            pso[:], V[:, c, ts(h, D)], ets[c][:],
                            start=(c == 0), stop=(c == NTC - 1),
                        )
                        if c < NTC // 2:
                            nc.tensor.matmul(
                                psz[:], ones[:], us[c][:],
                                start=(c == 0), stop=(c == NTC // 2 - 1),
                            )
                    rz = smp.tile([128, TB], F32, tag="rz")
                    nc.vector.reciprocal_approx_fast(rz[:], psz[:])
                    nc.vector.tensor_mul(oT[:, h, ts(ib, TB)], pso[:], rz[:])
                # output projection for this i-block (all 4 heads done)
                for cb in range(NTB):
                    wot = wop.tile([128, HL, TB], MM_DT, tag="wot", name=f"wot{ib}_{cb}")
                    nc.sync.dma_start(wot[:], wo_v[:, :, ts(cb, TB)])
                    for tto in range(NTB):
                        tt = ib * NTB + tto
                        ps = psum.tile(
                            [128, TB], F32, tag=f"pq{6 + tto % 2}",
                            name=f"psc{ib}_{cb}_{tto}",
                        )
                        for h in range(HL):
                            nc.tensor.matmul(
                                ps[:], oT[:, h, ts(tt, 128)], wot[:, h, :],
                                start=(h == 0), stop=(h == HL - 1),
                            )
                        ob = ocb.tile([128, TB], F32, tag="ob")
                        if tto % 2 == 0:
                            nc.vector.tensor_copy(ob[:], ps[:])
                        else:
                            nc.scalar.copy(ob[:], ps[:])
                        nc.sync.dma_start(out[ts(tt, 128), ts(cb, TB)], ob[:])

    nc.finalize()
    return nc


def _host_tables():
    inv_freq = (
        1.0 / (np.float32(ROPE_BASE) ** (np.arange(0, D, 2, dtype=np.float32) / np.float32(D)))
    ).astype(np.float32)
    t = np.arange(T, dtype=np.float32)
    freqs = (t[:, None] * inv_freq[None, :]).astype(np.float32)  # [T, 64]
    cos = np.cos(freqs).T.astype(np.float32)                     # [64, T]
    sin = np.sin(freqs).T.astype(np.float32)
    cosx = np.ascontiguousarray(np.concatenate([cos, cos], axis=0))      # [128, T]
    sinx = np.ascontiguousarray(np.concatenate([-sin, sin], axis=0))
    p = np.arange(128, dtype=np.int64)[:, None]
    u = np.arange(MASK_W, dtype=np.int64)[None, :]
    delta = u - MASK_OFF - p          # = i - j for tile offset
    allow = ~((delta >= 0) & (delta <= WINDOW - 1))
    maskm = np.ascontiguousarray(allow.astype(NP_MM))
    return cosx, sinx, maskm


def kernel(x, Wq, Wk, Wv, Wo):
    global _NC, LAST_RESULT
    if _NC is None:
        _NC = build_nc()
    x = np.asarray(x, dtype=np.float32)
    Wq = np.asarray(Wq, dtype=np.float32)
    Wk = np.asarray(Wk, dtype=np.float32)
    Wv = np.asarray(Wv, dtype=np.float32)
    Wo = np.asarray(Wo, dtype=np.float32)
    cosx, sinx, maskm = _host_tables()
    in_maps = []
    for core in range(NCORES):
        b, hg = divmod(core, NCORES // B)
        sl = slice(hg * HL * D, (hg + 1) * HL * D)
        in_maps.append(
            {
                "xT": np.ascontiguousarray(x[b].T.astype(NP_MM)),
                "wq": np.ascontiguousarray(Wq[:, sl].astype(NP_MM)),
                "wk": np.ascontiguousarray(Wk[:, sl].astype(NP_MM)),
                "wv": np.ascontiguousarray(Wv[:, sl].astype(NP_MM)),
                "wo": np.ascontiguousarray(Wo[sl, :].astype(NP_MM)),
                "cosx": cosx,
                "sinx": sinx,
                "maskm": maskm,
            }
        )
    res = run_bass_kernel_spmd(_NC, in_maps, list(range(NCORES)), trace=TRACE)
    LAST_RESULT = res
    out = np.zeros((B, T, C), dtype=np.float32)
    for core in range(NCORES):
        b = core // (NCORES // B)
        out[b] += res.results[core]["out"]
    return out
